# revision 2
# baseline (speedup 1.0000x reference)
"""Contextual-attention kernel for Trainium2 (8 NeuronCores via jax/axon).

Self-contained: accepts FULL inputs (f[4,96,128,128], b[4,96,128,128],
mask[1,1,128,128]) and returns the FULL output [4,96,128,128].

Sharding: pure data parallel over the batch dimension — each sample's
patch-correlation / fuse / softmax / deconv chain is independent, so
sample i runs on NeuronCore i. The per-sample graph is written entirely
with dense ops (strided slices, matmuls/einsums, slice-based
overlap-add) — no gather/scatter — so the neuron compiler can lower it.
Falls back to CPU execution if device compile/exec fails.
"""

import numpy as np

SCALE = 10.0


def _build():
    import jax
    import jax.numpy as jnp

    def down2(x):
        # nearest-neighbor resize 128->64 with align_corners=True:
        # indices are [0,2,...,62, 65,67,...,127] on each axis.
        a = jnp.concatenate([x[..., 0:64:2, :], x[..., 65:128:2, :]], axis=-2)
        return jnp.concatenate([a[..., 0:64:2], a[..., 65:128:2]], axis=-1)

    def diag_sum(t):
        # 3x3 identity-kernel SAME conv over a 2D grid:
        # out[r,c] = t[r-1,c-1] + t[r,c] + t[r+1,c+1] (zero padded)
        tp = jnp.pad(t, ((1, 1), (1, 1)))
        return tp[:-2, :-2] + tp[1:-1, 1:-1] + tp[2:, 2:]

    def one_sample(fi, bi, mask):
        # fi, bi: [C=96, 128, 128]; mask: [1, 1, 128, 128]
        C = fi.shape[0]
        fh = fw = bh = bw = 64
        L = bh * bw

        f_down = down2(fi)                  # [C, 64, 64]
        b_down = down2(bi)                  # [C, 64, 64]
        mask_down = down2(mask[0, 0])       # [64, 64]

        # --- background 3x3 patches of b_down (SAME, stride 1), as the
        # correlation weights: wp[l, c, dy, dx], l = by*bw + bx
        bp = jnp.pad(b_down, ((0, 0), (1, 1), (1, 1)))
        bsh = jnp.stack([bp[:, dy:dy + 64, dx:dx + 64]
                         for dy in range(3) for dx in range(3)], axis=0)
        # bsh: [9, C, by, bx] -> wp9[l, c*9+p] with p = dy*3+dx
        wp = bsh.transpose(2, 3, 1, 0).reshape(L, C * 9)   # [L, 864]
        norm = jnp.sqrt(jnp.sum(wp * wp, axis=1, keepdims=True))
        wn = wp / jnp.maximum(norm, 1e-4)                  # [L, 864]

        # --- foreground patch matrix: fp[c*9+p, y*fw+x]
        fpad = jnp.pad(f_down, ((0, 0), (1, 1), (1, 1)))
        fsh = jnp.stack([fpad[:, dy:dy + 64, dx:dx + 64]
                         for dy in range(3) for dx in range(3)], axis=1)
        fp = fsh.reshape(C * 9, fh * fw)                   # [864, 4096]

        # --- correlation scores: S[l, p] = <wn[l], f_patch[p]>
        S = wn @ fp                                        # [L, 4096]

        # --- mask: patch of mask_down fully outside hole -> 1
        mp = jnp.pad(mask_down, ((1, 1), (1, 1)))
        msh = sum(mp[dy:dy + 64, dx:dx + 64]
                  for dy in range(3) for dx in range(3))
        mm = (msh.reshape(L) == 0.0).astype(fi.dtype)      # [L]

        # --- fuse pass 1: over (y-major fg, by-major bg) flattened grid
        t = S.T                                            # [p=(y,x), l=(by,bx)]
        t = diag_sum(t)
        # pass 2: transpose to x-major / bx-major flattening
        t = t.reshape(fh, fw, bh, bw).transpose(1, 0, 3, 2).reshape(
            fw * fh, bw * bh)
        t = diag_sum(t)
        t = t.reshape(fw, fh, bw, bh).transpose(1, 0, 3, 2)  # [fh,fw,bh,bw]
        Sf = t.reshape(fh * fw, L).T                         # [l, p]

        # --- masked softmax over l
        logits = Sf * (mm[:, None] * SCALE)
        logits = logits - jnp.max(logits, axis=0, keepdims=True)
        e = jnp.exp(logits)
        A = e / jnp.sum(e, axis=0, keepdims=True)
        A = A * mm[:, None]                                  # [l, p]

        # --- deconv: raw 4x4 patches of full-res b (stride 2, SAME)
        bfp = jnp.pad(bi, ((0, 0), (1, 1), (1, 1)))          # [C, 130, 130]
        rsh = jnp.stack([bfp[:, i:i + 127:2, j:j + 127:2]
                         for i in range(4) for j in range(4)], axis=1)
        # rsh: [C, 16, by, bx] -> raw[l, c*16+ij]
        raw = rsh.transpose(2, 3, 0, 1).reshape(L, C * 16)   # [L, 1536]

        # G[c*16+ij, p] = sum_l raw[l, cij] * A[l, p]
        G = raw.T @ A                                        # [1536, 4096]
        G = G.reshape(C, 4, 4, fh, fw)                       # [c, i, j, y, x]

        # overlap-add rows: out row Y=2u+a <- i=a+1 (y=u), plus
        # a=0: i=3 (y=u-1);  a=1: i=0 (y=u+1)
        g3d = jnp.pad(G[:, 3, :, :-1, :], ((0, 0), (0, 0), (1, 0), (0, 0)))
        g0u = jnp.pad(G[:, 0, :, 1:, :], ((0, 0), (0, 0), (0, 1), (0, 0)))
        r_even = G[:, 1] + g3d                               # [c, j, u, x]
        r_odd = G[:, 2] + g0u
        M = jnp.stack([r_even, r_odd], axis=3).reshape(C, 4, 128, fw)

        # overlap-add cols: out col X=2v+a <- j=a+1 (x=v), plus
        # a=0: j=3 (x=v-1);  a=1: j=0 (x=v+1)
        m3d = jnp.pad(M[:, 3, :, :-1], ((0, 0), (0, 0), (1, 0)))
        m0u = jnp.pad(M[:, 0, :, 1:], ((0, 0), (0, 0), (0, 1)))
        c_even = M[:, 1] + m3d                               # [c, Y, v]
        c_odd = M[:, 2] + m0u
        out = jnp.stack([c_even, c_odd], axis=3).reshape(C, 128, 128)
        return out / 4.0

    return jax, one_sample


def kernel(f: np.ndarray, b: np.ndarray, mask: np.ndarray) -> np.ndarray:
    jax, one_sample = _build()

    f = np.asarray(f, dtype=np.float32)
    b = np.asarray(b, dtype=np.float32)
    mask = np.asarray(mask, dtype=np.float32)
    B = f.shape[0]

    jit_fn = jax.jit(one_sample)

    try:
        devs = [d for d in jax.devices() if d.platform != 'cpu']
        if not devs:
            raise RuntimeError("no accelerator devices")
        futs = []
        for i in range(B):
            d = devs[i % len(devs)]
            futs.append(jit_fn(jax.device_put(f[i], d),
                               jax.device_put(b[i], d),
                               jax.device_put(mask, d)))
        outs = [np.asarray(o) for o in futs]
    except Exception:
        cpu = jax.devices('cpu')[0]
        outs = []
        with jax.default_device(cpu):
            for i in range(B):
                outs.append(np.asarray(jit_fn(f[i], b[i], mask)))

    return np.stack(outs, axis=0).astype(np.float32)
